# revision 14
# baseline (speedup 1.0000x reference)
"""Trainium2 Bass kernel for nn_GNN_EBM (gnn_message_passing).

Math: the reference broadcasts one shared feature vector h0[b,:] to all
d_nodes graph nodes before message passing, and the adjacency
A = sigmoid(B_param) * mask is elementwise non-negative.  Hence

  conv1:  relu(h0*(1 + rowsum(A)_i/N))      = c_i * relu(h0)   (c_i > 0)
  conv2:  relu(r*(c_i + (A@c)_i/N))         = g_i * r          (r >= 0, g_i > 0)

so the whole GNN collapses to e = MLP_T(g_T * r) + MLP_Y(g_Y * r) with
r = relu(z @ fc_in_w.T + fc_in_b), and the scalars g_T, g_Y fold into the
MLP first-layer weights.  The device kernel is a fused 3-layer MLP over the
batch, data-parallel across 8 cores (256 rows/core), activations kept
transposed ([feature, batch]) so matmul outputs chain without transposes.

Device-side schedule (all latency-bound; per-op fixed costs dominate):
  - all matmul operands bf16 (single-pass PE matmuls, half the DMA bytes)
  - fc_in bias folded into the layer-1 matmul via a ones row (k=103), so
    the layer-1 relus are pure max-with-0 and need no bias operand
  - the critical za load is row-split across the two HWDGE engines
    (sync + scalar) so both descriptor streams run in parallel
  - a dummy activation at body start preloads the scalar engine's
    ACT table during the DMA arm window; activations then alternate
    scalar/vector so consecutive relu pairs overlap
"""

import sys

sys.path.insert(0, "/opt/trn_rl_repo")

import numpy as np
import ml_dtypes

import concourse.bacc as bacc
import concourse.mybir as mybir
import concourse.tile as tile
from concourse.bass_utils import run_bass_kernel_spmd


def _ensure_ntff_hook():
    # bass_utils' trace path imports antenv.axon_hooks, which some agent
    # images lack; register the ctypes-based hook ourselves so BASS_TRACE=1
    # yields an NTFF profile instead of an ImportError.
    try:
        import antenv.axon_hooks  # noqa: F401
        return
    except ImportError:
        pass
    import types

    import antenv

    mod = types.ModuleType("antenv.axon_hooks")
    holder = {"hook": None}
    mod.set_axon_ntff_profile_hook = lambda h: holder.__setitem__("hook", h)
    mod.get_axon_ntff_profile_hook = lambda: holder["hook"]
    sys.modules["antenv.axon_hooks"] = mod
    antenv.axon_hooks = mod
    try:
        from trn_agent_boot.trn_boot import _ntff_profile_via_ctypes

        hook = _ntff_profile_via_ctypes("/opt/axon/libaxon_pjrt.so")
        if hook is not None:
            mod.set_axon_ntff_profile_hook(hook)
    except Exception:
        pass


_ensure_ntff_hook()

N_CORES = 8
BATCH = 2048
D_X = 100
D_IN = D_X + 2             # x + t + y = 102
KD = D_IN + 1              # + ones row for the fc_in bias = 103
HID = 256
MLP_HID = 128
SHARD = BATCH // N_CORES   # 256

F32 = mybir.dt.float32
BF16 = mybir.dt.bfloat16
BF16_NP = ml_dtypes.bfloat16

_NC_CACHE = None
LAST_RESULT = None         # BassKernelResults of the most recent run


def _build_nc():
    nc = bacc.Bacc("TRN2", target_bir_lowering=False, debug=False,
                   num_devices=N_CORES)

    # za: [103, 512] = [ zT;ones (cols 0:256) | fc_in_w.T;fc_in_b (256:512) ]
    # wb: [128, 514] = [ w1 k-chunk0 | w1 k-chunk1 | w2T | w2Y ]  (bf16)
    # bb: [128, 4] fp32 = [ b1T | b1Y | b2 (row 0) | pad ]
    za_d = nc.dram_tensor("za", [KD, 512], BF16, kind="ExternalInput")
    wb_d = nc.dram_tensor("wb", [128, 514], BF16, kind="ExternalInput")
    bb_d = nc.dram_tensor("bb", [128, 4], F32, kind="ExternalInput")
    out_d = nc.dram_tensor("out", [1, SHARD], F32, kind="ExternalOutput")

    ADD = mybir.AluOpType.add
    MAX = mybir.AluOpType.max
    RELU = mybir.ActivationFunctionType.Relu
    IDENT = mybir.ActivationFunctionType.Identity

    with tile.TileContext(nc) as tc:
        with (
            tc.tile_pool(name="sb", bufs=1) as sb,
            tc.tile_pool(name="ps", bufs=1, space="PSUM") as ps,
        ):
            za = sb.tile([KD, 512], BF16, tag="za")
            wb = sb.tile([128, 514], BF16, tag="wb")
            bb = sb.tile([128, 4], F32, tag="bb")
            dum = sb.tile([128, 1], F32, tag="dum")

            # critical z/w_in load split across both HWDGE engines.
            # NOTE: HWDGE completion sems count per-engine, so anything
            # queued after za on the same engine delays za's waiters;
            # sync's queue carries ONLY za_lo (then the output much later),
            # scalar's queue carries za_hi -> wb -> bb in need order.
            nc.sync.dma_start(za[0:32, :], za_d[0:32, :])
            nc.scalar.dma_start(za[32:KD, :], za_d[32:KD, :])
            nc.scalar.dma_start(wb[:], wb_d[:])
            nc.scalar.dma_start(bb[:], bb_d[:])
            # dummy activation: forces the ACT table load to happen now,
            # hidden under the DMA arm latency
            nc.scalar.activation(dum[:], nc.const_aps.aps[(F32, 0.0)], RELU)

            # h^T = [fc_in_w | fc_in_b] @ [z^T ; 1], feature chunks of 128
            h0 = ps.tile([128, SHARD], F32, tag="h0")
            h1 = ps.tile([128, SHARD], F32, tag="h1")
            nc.tensor.matmul(h0[:], za[:, 256:384], za[:, 0:256])
            nc.tensor.matmul(h1[:], za[:, 384:512], za[:, 0:256])

            # r = relu(h), bias already folded in; split across vector/scalar
            r0 = sb.tile([128, SHARD], BF16, tag="r0")
            r1 = sb.tile([128, SHARD], BF16, tag="r1")
            nc.vector.tensor_scalar(r0[:], h0[:], 0.0, None, MAX)
            nc.scalar.activation(r1[:], h1[:], RELU)

            # u_head^T = (g_head * w1_head) @ r^T, two k-chunks accumulated
            uT = ps.tile([128, SHARD], F32, tag="uT")
            uY = ps.tile([128, SHARD], F32, tag="uY")
            # throwaway matmul: keeps the PE pstate ramped while waiting
            # for r0 (an idle PE drops to a ~1.7x slower state)
            warm = ps.tile([128, SHARD], F32, tag="warm")
            nc.tensor.matmul(warm[:], za[:, 256:384], za[:, 0:256])
            nc.tensor.matmul(uT[:], wb[:, 0:128], r0[:], start=True, stop=False)
            nc.tensor.matmul(uT[:], wb[:, 256:384], r1[:], start=False, stop=True)
            nc.tensor.matmul(uY[:], wb[:, 128:256], r0[:], start=True, stop=False)
            nc.tensor.matmul(uY[:], wb[:, 384:512], r1[:], start=False, stop=True)

            # s = relu(u + b1), bias+relu fused, scalar/vector in parallel
            sT = sb.tile([128, SHARD], BF16, tag="sT")
            sY = sb.tile([128, SHARD], BF16, tag="sY")
            nc.scalar.activation(sT[:], uT[:], RELU, bias=bb[:, 0:1])
            nc.vector.tensor_scalar(sY[:], uY[:], bb[:, 1:2], 0.0, ADD, MAX)

            # e = w2_T . s_T + w2_Y . s_Y + (b2_T + b2_Y)
            e = ps.tile([1, SHARD], F32, tag="e")
            nc.tensor.matmul(warm[:], za[:, 256:384], za[:, 0:256])
            nc.tensor.matmul(e[:], wb[:, 512:513], sT[:], start=True, stop=False)
            nc.tensor.matmul(e[:], wb[:, 513:514], sY[:], start=False, stop=True)

            # final copy+bias and the output DMA both on the scalar engine:
            # no cross-engine hop, and its HWDGE queue arms ~3x faster
            o = sb.tile([1, SHARD], F32, tag="o")
            nc.scalar.activation(o[:], e[:], IDENT, bias=bb[0:1, 2:3])
            nc.scalar.dma_start(out_d[:], o[:])

    nc.compile()
    return nc


def _get_nc():
    global _NC_CACHE
    if _NC_CACHE is None:
        _NC_CACHE = _build_nc()
    return _NC_CACHE


def kernel(**inputs: np.ndarray) -> np.ndarray:
    global LAST_RESULT
    x = np.asarray(inputs["x"], np.float32)
    t = np.asarray(inputs["t"], np.float32)
    y = np.asarray(inputs["y"], np.float32)
    B_param = np.asarray(inputs["B_param"], np.float32)
    fc_in_w = np.asarray(inputs["fc_in_w"], np.float32)
    fc_in_b = np.asarray(inputs["fc_in_b"], np.float32)
    eT_w1 = np.asarray(inputs["eT_w1"], np.float32)
    eT_b1 = np.asarray(inputs["eT_b1"], np.float32)
    eT_w2 = np.asarray(inputs["eT_w2"], np.float32)
    eT_b2 = np.asarray(inputs["eT_b2"], np.float32)
    eY_w1 = np.asarray(inputs["eY_w1"], np.float32)
    eY_b1 = np.asarray(inputs["eY_b1"], np.float32)
    eY_w2 = np.asarray(inputs["eY_w2"], np.float32)
    eY_b2 = np.asarray(inputs["eY_b2"], np.float32)

    # collapse the two message-passing layers to per-node scalars
    n = B_param.shape[0]
    mask = np.ones((n, n), np.float32)
    mask[-1, :D_X] = 0.0
    np.fill_diagonal(mask, 0.0)
    A = mask / (1.0 + np.exp(-B_param))
    c = 1.0 + A.sum(axis=1) / n
    g = c + (A @ c) / n
    gT, gY = np.float32(g[n - 2]), np.float32(g[n - 1])

    w1cat = np.concatenate([gT * eT_w1.T, gY * eY_w1.T], axis=1)  # [256, 256]
    wb_arr = np.zeros((128, 514), BF16_NP)
    wb_arr[:, 0:256] = w1cat[0:128].astype(BF16_NP)
    wb_arr[:, 256:512] = w1cat[128:256].astype(BF16_NP)
    wb_arr[:, 512] = eT_w2[0].astype(BF16_NP)
    wb_arr[:, 513] = eY_w2[0].astype(BF16_NP)

    bb_arr = np.zeros((128, 4), np.float32)
    bb_arr[:, 0] = eT_b1
    bb_arr[:, 1] = eY_b1
    bb_arr[0, 2] = eT_b2[0] + eY_b2[0]

    z = np.concatenate([x, t, y], axis=1)  # [BATCH, 102]
    wcol = np.empty((KD, 256), np.float32)  # [fc_in_w.T ; fc_in_b]
    wcol[0:D_IN] = fc_in_w.T
    wcol[D_IN] = fc_in_b
    wcol_bf = wcol.astype(BF16_NP)
    in_maps = []
    for i in range(N_CORES):
        za_arr = np.empty((KD, 512), BF16_NP)
        za_arr[0:D_IN, 0:256] = z[i * SHARD:(i + 1) * SHARD].T.astype(BF16_NP)
        za_arr[D_IN, 0:256] = BF16_NP(1.0)
        za_arr[:, 256:512] = wcol_bf
        in_maps.append({"za": za_arr, "wb": wb_arr, "bb": bb_arr})

    nc = _get_nc()
    LAST_RESULT = run_bass_kernel_spmd(nc, in_maps, list(range(N_CORES)))
    return np.concatenate(
        [r["out"].reshape(SHARD) for r in LAST_RESULT.results]
    ).astype(np.float32)


# revision 19
# speedup vs baseline: 1.1362x; 1.1362x over previous
"""Trainium2 Bass kernel for nn_GNN_EBM (gnn_message_passing).

Math: the reference broadcasts one shared feature vector h0[b,:] to all
d_nodes graph nodes before message passing, and the adjacency
A = sigmoid(B_param) * mask is elementwise non-negative.  Hence

  conv1:  relu(h0*(1 + rowsum(A)_i/N))      = c_i * relu(h0)   (c_i > 0)
  conv2:  relu(r*(c_i + (A@c)_i/N))         = g_i * r          (r >= 0, g_i > 0)

so the whole GNN collapses to e = MLP_T(g_T * r) + MLP_Y(g_Y * r) with
r = relu(z @ fc_in_w.T + fc_in_b), and the scalars g_T, g_Y fold into the
MLP first-layer weights.  The device kernel is a fused 3-layer MLP over the
batch, data-parallel across 8 cores (256 rows/core), activations kept
transposed ([feature, batch]) so matmul outputs chain without transposes.

Device-side schedule (all latency-bound; per-op fixed costs dominate):
  - all matmul operands bf16 (single-pass PE matmuls, half the DMA bytes)
  - fc_in bias folded into the layer-1 matmul via a ones row (k=103), so
    the layer-1 relus are pure max-with-0 and need no bias operand
  - the critical za load is row-split across the two HWDGE engines
    (sync + scalar) so both descriptor streams run in parallel
  - a dummy activation at body start preloads the scalar engine's
    ACT table during the DMA arm window; activations then alternate
    scalar/vector so consecutive relu pairs overlap
"""

import sys

sys.path.insert(0, "/opt/trn_rl_repo")

import numpy as np
import ml_dtypes

import concourse.bacc as bacc
import concourse.mybir as mybir
import concourse.tile as tile
from concourse.bass_utils import run_bass_kernel_spmd


def _ensure_ntff_hook():
    # bass_utils' trace path imports antenv.axon_hooks, which some agent
    # images lack; register the ctypes-based hook ourselves so BASS_TRACE=1
    # yields an NTFF profile instead of an ImportError.
    try:
        import antenv.axon_hooks  # noqa: F401
        return
    except ImportError:
        pass
    import types

    import antenv

    mod = types.ModuleType("antenv.axon_hooks")
    holder = {"hook": None}
    mod.set_axon_ntff_profile_hook = lambda h: holder.__setitem__("hook", h)
    mod.get_axon_ntff_profile_hook = lambda: holder["hook"]
    sys.modules["antenv.axon_hooks"] = mod
    antenv.axon_hooks = mod
    try:
        from trn_agent_boot.trn_boot import _ntff_profile_via_ctypes

        hook = _ntff_profile_via_ctypes("/opt/axon/libaxon_pjrt.so")
        if hook is not None:
            mod.set_axon_ntff_profile_hook(hook)
    except Exception:
        pass


_ensure_ntff_hook()

N_CORES = 8
BATCH = 2048
D_X = 100
D_IN = D_X + 2             # x + t + y = 102
KD = D_IN + 1              # + ones row for the fc_in bias = 103
HID = 256
MLP_HID = 128
SHARD = BATCH // N_CORES   # 256

F32 = mybir.dt.float32
BF16 = mybir.dt.bfloat16
BF16_NP = ml_dtypes.bfloat16

_NC_CACHE = None
LAST_RESULT = None         # BassKernelResults of the most recent run


def _build_nc():
    nc = bacc.Bacc("TRN2", target_bir_lowering=False, debug=False,
                   num_devices=N_CORES)

    # za: [103, 512] = [ zT;ones (cols 0:256) | fc_in_w.T;fc_in_b (256:512) ]
    # wb: [128, 514] = [ w1 k-chunk0 | w1 k-chunk1 | w2T | w2Y ]  (bf16)
    # b1d: [1, 513] = [ ones | b1T | b1Y | b2 ] -- one DMA descriptor; applied as
    #     k=1 matmuls against the ones row so no per-partition bias
    #     operand (and no fp32 side tensor) is needed anywhere
    za_d = nc.dram_tensor("za", [KD, 512], BF16, kind="ExternalInput")
    wb_d = nc.dram_tensor("wb", [128, 514], BF16, kind="ExternalInput")
    b1_d = nc.dram_tensor("b1d", [1, 513], BF16, kind="ExternalInput")
    out_d = nc.dram_tensor("out", [1, SHARD], F32, kind="ExternalOutput")

    ADD = mybir.AluOpType.add
    MAX = mybir.AluOpType.max
    RELU = mybir.ActivationFunctionType.Relu
    IDENT = mybir.ActivationFunctionType.Identity

    with tile.TileContext(nc) as tc:
        with (
            tc.tile_pool(name="sb", bufs=1) as sb,
            tc.tile_pool(name="ps", bufs=1, space="PSUM") as ps,
        ):
            za = sb.tile([KD, 512], BF16, tag="za")
            wb = sb.tile([128, 514], BF16, tag="wb")
            b1 = sb.tile([1, 513], BF16, tag="b1")
            dum = sb.tile([128, 1], F32, tag="dum")

            # critical z/w_in load split across both HWDGE engines.
            # NOTE: HWDGE completion sems count per-engine-queue, so each
            # queue carries transfers strictly in need order.
            nc.sync.dma_start(za[0:52, :], za_d[0:52, :])
            nc.scalar.dma_start(za[52:KD, :], za_d[52:KD, :])
            nc.scalar.dma_start(wb[:], wb_d[:])
            nc.sync.dma_start(b1[:], b1_d[:])
            # dummy activation: forces the ACT table load to happen now,
            # hidden under the DMA arm latency
            nc.scalar.activation(dum[:], nc.const_aps.aps[(F32, 0.0)], RELU)

            # h^T = [fc_in_w | fc_in_b] @ [z^T ; 1], feature chunks of 128
            h0 = ps.tile([128, SHARD], F32, tag="h0")
            h1 = ps.tile([128, SHARD], F32, tag="h1")
            nc.tensor.matmul(h0[:], za[:, 256:384], za[:, 0:256])
            nc.tensor.matmul(h1[:], za[:, 384:512], za[:, 0:256])

            # r = relu(h), bias already folded in; split across vector/scalar
            r0 = sb.tile([128, SHARD], BF16, tag="r0")
            r1 = sb.tile([128, SHARD], BF16, tag="r1")
            nc.vector.tensor_scalar(r0[:], h0[:], 0.0, None, MAX)
            nc.scalar.activation(r1[:], h1[:], RELU)

            # u_head^T = (g_head * w1_head) @ r^T + b1_head.  The k=1 bias
            # matmuls (b1 row x ones row) start each accumulation group;
            # they are dependency-free so they also keep the PE pstate
            # ramped while waiting for r0.
            ones = b1[0:1, 0:256]
            uT = ps.tile([128, SHARD], F32, tag="uT")
            uY = ps.tile([128, SHARD], F32, tag="uY")
            nc.tensor.matmul(uT[:], b1[0:1, 256:384], ones, start=True, stop=False)
            nc.tensor.matmul(uY[:], b1[0:1, 384:512], ones, start=True, stop=False)
            nc.tensor.matmul(uT[:], wb[:, 0:128], r0[:], start=False, stop=False)
            nc.tensor.matmul(uT[:], wb[:, 256:384], r1[:], start=False, stop=True)
            nc.tensor.matmul(uY[:], wb[:, 128:256], r0[:], start=False, stop=False)
            nc.tensor.matmul(uY[:], wb[:, 384:512], r1[:], start=False, stop=True)

            # s = relu(u), bias already in PSUM; scalar/vector in parallel
            sT = sb.tile([128, SHARD], BF16, tag="sT")
            sY = sb.tile([128, SHARD], BF16, tag="sY")
            nc.scalar.activation(sT[:], uT[:], RELU)
            nc.vector.tensor_scalar(sY[:], uY[:], 0.0, None, MAX)

            # e = w2_T . s_T + w2_Y . s_Y + b2, b2 via ones again
            e = ps.tile([1, SHARD], F32, tag="e")
            nc.tensor.matmul(e[:], b1[0:1, 512:513], ones, start=True, stop=False)
            nc.tensor.matmul(e[:], wb[:, 512:513], sT[:], start=False, stop=False)
            nc.tensor.matmul(e[:], wb[:, 513:514], sY[:], start=False, stop=True)

            # final copy and the output DMA both on the scalar engine:
            # no cross-engine hop, and its HWDGE queue arms ~3x faster
            o = sb.tile([1, SHARD], F32, tag="o")
            nc.scalar.activation(o[:], e[:], IDENT)
            nc.scalar.dma_start(out_d[:], o[:])

    nc.compile()
    return nc


def _get_nc():
    global _NC_CACHE
    if _NC_CACHE is None:
        _NC_CACHE = _build_nc()
    return _NC_CACHE


def kernel(**inputs: np.ndarray) -> np.ndarray:
    global LAST_RESULT
    x = np.asarray(inputs["x"], np.float32)
    t = np.asarray(inputs["t"], np.float32)
    y = np.asarray(inputs["y"], np.float32)
    B_param = np.asarray(inputs["B_param"], np.float32)
    fc_in_w = np.asarray(inputs["fc_in_w"], np.float32)
    fc_in_b = np.asarray(inputs["fc_in_b"], np.float32)
    eT_w1 = np.asarray(inputs["eT_w1"], np.float32)
    eT_b1 = np.asarray(inputs["eT_b1"], np.float32)
    eT_w2 = np.asarray(inputs["eT_w2"], np.float32)
    eT_b2 = np.asarray(inputs["eT_b2"], np.float32)
    eY_w1 = np.asarray(inputs["eY_w1"], np.float32)
    eY_b1 = np.asarray(inputs["eY_b1"], np.float32)
    eY_w2 = np.asarray(inputs["eY_w2"], np.float32)
    eY_b2 = np.asarray(inputs["eY_b2"], np.float32)

    # collapse the two message-passing layers to per-node scalars
    n = B_param.shape[0]
    mask = np.ones((n, n), np.float32)
    mask[-1, :D_X] = 0.0
    np.fill_diagonal(mask, 0.0)
    A = mask / (1.0 + np.exp(-B_param))
    c = 1.0 + A.sum(axis=1) / n
    g = c + (A @ c) / n
    gT, gY = np.float32(g[n - 2]), np.float32(g[n - 1])

    w1cat = np.concatenate([gT * eT_w1.T, gY * eY_w1.T], axis=1)  # [256, 256]
    wb_arr = np.zeros((128, 514), BF16_NP)
    wb_arr[:, 0:256] = w1cat[0:128].astype(BF16_NP)
    wb_arr[:, 256:512] = w1cat[128:256].astype(BF16_NP)
    wb_arr[:, 512] = eT_w2[0].astype(BF16_NP)
    wb_arr[:, 513] = eY_w2[0].astype(BF16_NP)

    b1_arr = np.empty((1, 513), BF16_NP)
    b1_arr[0, 0:256] = BF16_NP(1.0)
    b1_arr[0, 256:384] = eT_b1.astype(BF16_NP)
    b1_arr[0, 384:512] = eY_b1.astype(BF16_NP)
    b1_arr[0, 512] = BF16_NP(eT_b2[0] + eY_b2[0])

    z = np.concatenate([x, t, y], axis=1)  # [BATCH, 102]
    wcol = np.empty((KD, 256), np.float32)  # [fc_in_w.T ; fc_in_b]
    wcol[0:D_IN] = fc_in_w.T
    wcol[D_IN] = fc_in_b
    wcol_bf = wcol.astype(BF16_NP)
    in_maps = []
    for i in range(N_CORES):
        za_arr = np.empty((KD, 512), BF16_NP)
        za_arr[0:D_IN, 0:256] = z[i * SHARD:(i + 1) * SHARD].T.astype(BF16_NP)
        za_arr[D_IN, 0:256] = BF16_NP(1.0)
        za_arr[:, 256:512] = wcol_bf
        in_maps.append({"za": za_arr, "wb": wb_arr, "b1d": b1_arr})

    nc = _get_nc()
    LAST_RESULT = run_bass_kernel_spmd(nc, in_maps, list(range(N_CORES)))
    return np.concatenate(
        [r["out"].reshape(SHARD) for r in LAST_RESULT.results]
    ).astype(np.float32)
